# revision 12
# baseline (speedup 1.0000x reference)
"""Trainium2 Bass kernel for CenterWoParamMultiCosineNearLoss.

loss = mean_b [ S_b - m_b + (2*m_b^2 - Q_b) / S_b ]   where, per sample b,
  d_k = 1 - <x_b, c_{label_b, k}>  (k = 0..15 sub-centers of own class)
  S = sum_k d_k, Q = sum_k d_k^2, m = min_k d_k
(algebraically identical to the reference's term1+term2).

In cos-space with s_k = <x_b, c_k>: T = sum s_k, U = sum s_k^2, M = max s_k
gives S = K - T, Q = K - 2T + U, m = 1 - M.

Sharding: samples sorted by label on the host (loss is permutation-invariant),
split into 8 contiguous shards of 1024 — data-parallel with class-clustered
assignment. Each 128-row block then spans only a handful of consecutive
classes, so its matmul streams a narrow per-block column window (wb classes,
wb*16 columns) of the core's transposed-centers window instead of the full
90*16 columns; with fp8 weights the auto fast-weight-load keeps the tensor
engine at ~54ns per 128x128x64 matmul. Per-block window offsets are
compile-time constants computed as a cross-core union (one SPMD program
serves all cores).

Selection of the 16 own-class columns per row uses a host-built one-hot mask
(value 1/128 = the fp8-scale descale; DMA'd together with the centers window
as one fp8 tensor). DVE work is batched over 4-block groups to amortize the
~100ns/instruction overhead:
  sm = psum * mask   (= cos, zeroed off-class)        [vector, one tt]
  T  = reduce_add(sm)  per block                      [vector]
  M  = reduce_max(sm)  per block                      [vector]
  U  = reduce_add(Square(sm))      [scalar/ACT engine + vector reduce]
(M=max(sm) clamps at 0 for rows whose 16 selected cos are all negative —
p ~ 2^-16 per row; error ~1e-5 on one row's loss, ~1e-9 on the mean.)

All inputs travel as fp8e4m3 (x scaled by 16, centers by 8). DMA triggers
cost ~0.6us each on the issuing sequencer, so there are only 3 input
transfers, issued on sync in consumption order: centers+mask, then two x
chunks of 4 blocks each. Each core emits its partial row-loss sum as a [1,1]
tensor; the host reduces the 8 scalars into the mean.
"""

import os
import sys

import numpy as np
import ml_dtypes

for _p in ("/opt/trn_rl_repo", "/root/.axon_site/_ro/trn_rl_repo"):
    if os.path.isdir(_p) and _p not in sys.path:
        sys.path.append(_p)

import concourse.tile as tile  # noqa: E402
from concourse import bacc  # noqa: E402
from concourse import mybir  # noqa: E402
from concourse.bass_utils import run_bass_kernel_spmd  # noqa: E402

P = 128          # SBUF partitions
B = 8192         # batch
D = 1024         # feature dim
C = 90           # classes
K = 16           # sub-centers per class
NCORES = 8
SHARD = B // NCORES          # 1024 samples per core
NB = SHARD // P              # 8 row-blocks per core
KT = D // P                  # 8 contraction tiles
NCH = 2                      # x DMA chunks per core
BPC = NB // NCH              # row-blocks per chunk (4)

_F32 = mybir.dt.float32
_F8 = mybir.dt.float8e4
_FP8NP = ml_dtypes.float8_e4m3

_ADD = mybir.AluOpType.add
_MULT = mybir.AluOpType.mult
_SUB = mybir.AluOpType.subtract
_MAX = mybir.AluOpType.max
_AX = mybir.AxisListType.X
_ACT = mybir.ActivationFunctionType

XS = 16.0                    # host scale on x (keeps fp8 in normal range)
CS = 8.0                     # host scale on centers
INV = 1.0 / (XS * CS)        # mask value: folds the descale into the select


def _build_program(w: int, wb: int, offs: list[int]):
    """One SPMD program for all 8 cores.

    w: window width in classes; wb: per-block sub-window width in classes;
    offs[i]: class offset of block i's sub-window inside the window
    (identical across cores by construction).
    """
    wc = w * K
    wbk = wb * K
    moff = KT * wc               # mask region start inside cwm
    cwmw = moff + NB * wbk

    nc = bacc.Bacc(None, target_bir_lowering=False)
    xT = nc.declare_dram_parameter("xT", [NCH, P, KT, BPC, P], _F8, isOutput=False)
    cwm = nc.declare_dram_parameter("cwm", [P, cwmw], _F8, isOutput=False)
    out = nc.declare_dram_parameter("out", [1, 1], _F32, isOutput=True)

    with tile.TileContext(nc) as tc:
        with (
            tc.tile_pool(name="const", bufs=1) as const,
            tc.tile_pool(name="cwmp", bufs=1) as cwmp,
            tc.tile_pool(name="xp", bufs=NCH) as xp,
            tc.tile_pool(name="work", bufs=2) as work,
            tc.tile_pool(name="stats", bufs=1) as stats,
            tc.tile_pool(name="pp", bufs=2, space="PSUM") as pp,
            tc.tile_pool(name="ppf", bufs=1, space="PSUM") as ppf,
        ):
            ones = const.tile([P, 1], _F32)
            nc.vector.memset(ones[:, :], 1.0)
            # 3 input transfers on sync, in consumption order
            cwmt = cwmp.tile([P, cwmw], _F8)
            nc.sync.dma_start(out=cwmt[:, :], in_=cwm[:, :])
            xts = []
            for ch in range(NCH):
                xt = xp.tile([P, KT, BPC, P], _F8, tag="xc")
                nc.sync.dma_start(out=xt[:, :, :, :], in_=xT[ch, :, :, :, :])
                xts.append(xt)

            Tt = stats.tile([P, NB], _F32)
            Ut = stats.tile([P, NB], _F32)
            Mt = stats.tile([P, NB], _F32)

            for ch in range(NCH):
                ps = pp.tile([P, BPC, wbk], _F32)
                for h in range(BPC):
                    i = ch * BPC + h
                    cbase = offs[i] * K
                    for kt in range(KT):
                        nc.tensor.matmul(
                            ps[:, h, :],
                            lhsT=xts[ch][:, kt, h, :],
                            rhs=cwmt[:, kt * wc + cbase : kt * wc + cbase + wbk],
                            start=(kt == 0),
                            stop=(kt == KT - 1),
                        )
                lo = ch * BPC
                # sm = cos over the 4 block windows, zeroed off-class
                sm = work.tile([P, BPC, wbk], _F32, tag="sm")
                nc.vector.tensor_tensor(
                    out=sm[:, :, :], in0=ps[:, :, :],
                    in1=cwmt[:, moff + lo * wbk : moff + (lo + BPC) * wbk].rearrange(
                        "p (b c) -> p b c", b=BPC
                    ),
                    op=_MULT,
                )
                nc.vector.tensor_reduce(
                    out=Tt[:, lo : lo + BPC], in_=sm[:, :, :], axis=_AX, op=_ADD,
                )
                nc.vector.tensor_reduce(
                    out=Mt[:, lo : lo + BPC], in_=sm[:, :, :], axis=_AX, op=_MAX,
                )
                squ = work.tile([P, BPC, wbk], _F32, tag="squ")
                nc.scalar.activation(
                    out=squ[:, :, :], in_=sm[:, :, :], func=_ACT.Square,
                )
                nc.vector.tensor_reduce(
                    out=Ut[:, lo : lo + BPC], in_=squ[:, :, :], axis=_AX, op=_ADD,
                )

            # epilogue on [P, NB]:
            #   S = K - T; m = 1 - M; 2m^2 - Q = 2m^2 + 2T - U - K
            #   rowloss = S - m + (2*m^2 - Q) / S
            # scalar computes m and 2m^2 in parallel with vector
            md = stats.tile([P, NB], _F32)
            nc.scalar.activation(
                out=md[:, :], in_=Mt[:, :], func=_ACT.Copy, bias=1.0, scale=-1.0,
            )
            num = stats.tile([P, NB], _F32)
            nc.scalar.activation(
                out=num[:, :], in_=md[:, :], func=_ACT.Square, scale=1.41421356237,
            )
            S = stats.tile([P, NB], _F32)
            nc.vector.tensor_scalar(
                out=S[:, :], in0=Tt[:, :], scalar1=-1.0, scalar2=float(K),
                op0=_MULT, op1=_ADD,
            )
            rs = stats.tile([P, NB], _F32)
            nc.vector.reciprocal(rs[:, :], S[:, :])
            A = stats.tile([P, NB], _F32)
            nc.vector.scalar_tensor_tensor(
                out=A[:, :], in0=Tt[:, :], scalar=2.0, in1=Ut[:, :],
                op0=_MULT, op1=_SUB,
            )
            AK = stats.tile([P, NB], _F32)
            nc.vector.tensor_scalar(
                out=AK[:, :], in0=A[:, :], scalar1=-float(K), scalar2=None, op0=_ADD,
            )
            numq = stats.tile([P, NB], _F32)
            nc.vector.tensor_tensor(out=numq[:, :], in0=num[:, :], in1=AK[:, :], op=_ADD)
            frac = stats.tile([P, NB], _F32)
            nc.vector.tensor_tensor(out=frac[:, :], in0=numq[:, :], in1=rs[:, :], op=_MULT)
            base = stats.tile([P, NB], _F32)
            nc.vector.tensor_tensor(out=base[:, :], in0=S[:, :], in1=md[:, :], op=_SUB)
            rl = stats.tile([P, NB], _F32)
            nc.vector.tensor_tensor(out=rl[:, :], in0=base[:, :], in1=frac[:, :], op=_ADD)
            rowsum = stats.tile([P, 1], _F32)
            nc.vector.tensor_reduce(out=rowsum[:, :], in_=rl[:, :], axis=_AX, op=_ADD)
            # cross-partition sum via ones-matmul -> single 4B output packet
            psc = ppf.tile([1, 1], _F32)
            nc.tensor.matmul(
                psc[:, :], lhsT=rowsum[:, :], rhs=ones[:, :], start=True, stop=True
            )
            outsb = stats.tile([1, 1], _F32)
            nc.vector.tensor_copy(out=outsb[:, :], in_=psc[:, :])
            nc.sync.dma_start(out=out[:, :], in_=outsb[:, :])

    nc.finalize()
    return nc


def _prep_inputs(x, labels, centers):
    """Host-side sharding/layout prep. Returns (in_maps, (w, wb, offs))."""
    labels = np.asarray(labels).astype(np.int64)
    x = np.ascontiguousarray(np.asarray(x, dtype=np.float32))
    centers = np.asarray(centers, dtype=np.float32)

    perm = np.argsort(labels, kind="stable")
    ls = labels[perm]

    # per-core local labels and per-block class spans
    labloc = np.empty((NCORES, SHARD), dtype=np.int64)
    starts = []
    for k in range(NCORES):
        seg = ls[k * SHARD : (k + 1) * SHARD]
        starts.append(int(seg[0]))
        labloc[k] = seg - seg[0]
    cb = labloc[:, ::P][:, :NB]                 # [NCORES, NB] first class in block
    ce = labloc[:, P - 1 :: P][:, :NB]          # [NCORES, NB] last class in block
    offs = cb.min(axis=0).astype(np.int64)      # cross-core union offsets
    wb = int((ce - offs[None, :]).max() + 1)
    w = int(max(labloc.max() + 1, (offs + wb).max()))
    wbk = wb * K
    assert wbk * BPC * 4 <= 2048, f"chunk window {BPC}x{wb} too large for a PSUM bank"

    # transposed, scaled, zero-padded centers: [D, w*K] class-major
    cwin_all = np.zeros((NCORES, D, w * K), dtype=np.float32)
    for k in range(NCORES):
        hi = min(starts[k] + w, C)
        ww = hi - starts[k]
        blk = centers[starts[k] : hi]                        # [ww, K, D]
        cwin_all[k, :, : ww * K] = (CS * blk).reshape(ww * K, D).T

    in_maps = []
    col_cls = (np.arange(wbk) // K).astype(np.int64)         # class id per column
    for k in range(NCORES):
        rows = perm[k * SHARD : (k + 1) * SHARD]
        xq = (XS * x[rows]).astype(_FP8NP)                   # [SHARD, D]
        # [NCH, P, KT, BPC, P]: partition p holds d = kt*128 + p
        xdev = np.ascontiguousarray(
            xq.T.reshape(KT, P, NCH, BPC * P).transpose(2, 1, 0, 3)
        ).reshape(NCH, P, KT, BPC, P)
        cw8 = cwin_all[k].reshape(KT, P, w * K).transpose(1, 0, 2).astype(_FP8NP)
        sub = labloc[k].reshape(NB, P).T - offs[None, :]     # [P, NB] in [0, wb)
        assert sub.min() >= 0 and sub.max() < wb
        onehot = sub[:, :, None] == col_cls[None, None, :]   # [P, NB, wbk]
        mask = (INV * onehot).astype(_FP8NP)
        cwm_dev = np.ascontiguousarray(
            np.concatenate(
                [cw8.reshape(P, KT * w * K), mask.reshape(P, NB * wbk)], axis=1
            )
        )
        in_maps.append({"xT": xdev, "cwm": cwm_dev})
    return in_maps, (w, wb, [int(o) for o in offs])


def kernel(x, labels, centers):
    in_maps, (w, wb, offs) = _prep_inputs(x, labels, centers)
    nc = _build_program(w, wb, offs)
    res = run_bass_kernel_spmd(nc, in_maps, core_ids=list(range(NCORES)))
    total = sum(float(r["out"].astype(np.float64).sum()) for r in res.results)
    return np.float32(total / B)
